# revision 48
# baseline (speedup 1.0000x reference)
"""DGL capsule routing layer on 8 trn2 NeuronCores (Bass/Tile), v2.

Math per pass t (b is linear in uh, so b_t = uh . w_t with w_t = cumsum v):
    b[i,o] = sum_f uh[i,o,f] * w[o,f]
    e = exp(b - 3); den[i] = sum_o e  (AllReduce over o-shards)
    chat[i,o] = e * (1/den)
    s[o,f] = sum_i chat[i,o] * uh[i,o,f]; v = squash(s); w += v
Pass 1 has chat uniform = 1/OUT.

Sharding: OUT_NODES split across 8 cores (128 local o per core). Each core
holds its full uh shard [4096 i, 128 o, 16 f] in SBUF as fp16 (host converts,
upload not counted in HW time). Per-core per pass:
  sweep1 (passes>=2): per block of 128 i: tm = uh*w_rep (DVE/GpSimd TT fp16),
    b = reduce_f (DVE, f32), e = exp(b-3) on ACT with fused den accum.
    den partials AllReduce'd in two halves so the first half's matmuls
    overlap the second half's sweep1.
  sweep2: chat_blk = e_blk * rinv (DVE tensor_scalar, fp16) becomes the PE
    STATIONARY [128 i, 128 o]; moving = uh block [128, 2048] -> psum
    [128 o, 2048 (o,f)] accumulated over all 32 blocks. The wanted s[o,f]
    is the "diagonal" ps[o, 16o+f], extracted via a DRAM round-trip with a
    stride-2080 read AP. squash on [128 o, 16] layout, w broadcast via DRAM.
Output: core c returns v slice [128, 16]; host concatenates.
"""

import numpy as np
from contextlib import ExitStack

import concourse.bass as bass
import concourse.mybir as mybir
import concourse.tile as tile
from concourse import bacc
from concourse import bass_utils

F32 = mybir.dt.float32
F16 = mybir.dt.float16
AX = mybir.AxisListType
AF = mybir.ActivationFunctionType

IN_NODES, OUT_NODES, F_SIZE = 4096, 1024, 16
CORES = 8
O_LOC = OUT_NODES // CORES         # 128 local out-nodes per core
P = 128
NBLK = IN_NODES // P               # 32 i-blocks per core
RB = O_LOC * F_SIZE                # 2048 elems per i-row (128 o x 16 f)
DVE_STB = 4                        # sweep1 supertile = 4 blocks, all on DVE
                                   # (GpSimd TT steals the shared SBUF port
                                   # and DVE TT drops to ~1/8 rate when they
                                   # overlap -- measured; so no GpSimd tms)
EXP_BIAS = -3.0                    # exp(b-3): keeps e in fp16 range
MMW = 512                          # matmul moving width (one psum bank)
U32 = mybir.dt.uint32
ALU = mybir.AluOpType


def _mm(nc, out, lhsT, rhs, start, stop, load):
    return nc.tensor.matmul(out, lhsT, rhs, start=start, stop=stop,
                            skip_group_check=True)


def _ap_key(lowered):
    try:
        return str(lowered)
    except Exception:
        return repr(lowered)


def _dedupe_ldweights(nc):
    """Drop InstLdweights whose stationary AP equals the previous PE-queue
    load (the tile layer emits one per matmul; repeated stationaries keep
    the array contents valid). Runs after TileContext exit, before
    nc.compile(). Waits on a dropped load are merged into the next one's
    consumer matmul implicitly (observed: duplicate loads carry no syncs)."""
    PE = mybir.EngineType.PE
    for b in nc.main_func.blocks:
        out = []
        last_key = None
        for ins in b.instructions:
            if getattr(ins, "engine", None) != PE:
                out.append(ins)
                continue
            if isinstance(ins, mybir.InstLdweights):
                key = _ap_key(ins.ins[0])
                si = ins.sync_info
                has_sync = si is not None and (si.on_wait or si.on_update)
                if key == last_key and not has_sync:
                    continue
                last_key = key
                out.append(ins)
            elif isinstance(ins, mybir.InstMatmult):
                out.append(ins)
            else:
                last_key = None
                out.append(ins)
    # noqa: the slice assignment below replaces the list in place
        b.instructions[:] = out

# DRAM scratch for the diagonal extract: write rows at stride 2064,
# read the diagonal (o, 16o+f) at stride 2080.
WSTR = RB + F_SIZE                 # 2064
DSTR = RB + 2 * F_SIZE             # 2080
SDUMP = P * DSTR                   # 266240 floats


HB = NBLK // 2                     # blocks per AR half


def _sweep2_half(nc, small, pspool, s_ps, uh_sb, e_all, rinv_all,
                 den_out, h):
    """chat = e * (1/den) for half h. Everything here runs on ACT + PE +
    SP only, so it overlaps the other half's sweep1 on DVE. rinv is
    computed as exp(-ln(den)) on ACT (both in the preloaded table set)."""
    dsum = small.tile([P, HB], F32, tag=f"dsum{h}")
    nc.sync.dma_start(dsum, den_out.rearrange("(p q) -> p q", p=P))
    lnd = small.tile([P, HB], F32, tag=f"lnd{h}")
    nc.scalar.activation(lnd, dsum, AF.Ln)
    rinv_h = rinv_all[:, h * HB:(h + 1) * HB]
    nc.scalar.activation(rinv_h, lnd, AF.Exp, scale=-1.0)
    for k in range(HB):
        blk = h * HB + k
        ch = small.tile([P, P], F16, tag="ch", bufs=4)
        nc.scalar.mul(ch, e_all[:, blk * P:(blk + 1) * P],
                      rinv_all[:, blk:blk + 1])
        if k == 0:
            # HAM warm-up: ~4.5us of N=512 matmuls gated on this half's
            # first chat, so the PE clock is at 2.4 GHz for the real MMs
            warm_ps = pspool.tile([P, MMW], F32, tag="warm_ps")
            for w in range(10):
                _mm(nc, warm_ps, ch, uh_sb[:, 0:MMW],
                    start=True, stop=True, load=(w == 0))
        for c in range(RB // MMW):
            _mm(nc, s_ps[:, c * MMW:(c + 1) * MMW],
                ch,
                uh_sb[:, blk * RB + c * MMW:blk * RB + (c + 1) * MMW],
                start=(blk == 0), stop=(blk == NBLK - 1), load=(c == 0))


def _squash_and_w(nc, small, w_rep, s_sb, w_acc, t, R, v_out, dram):
    """s_sb [128 o, 16] f32 -> v; if not last pass, w_acc += v and broadcast
    w to w_rep [128, 2048] fp16 via DRAM."""
    ssq = small.tile([P, F_SIZE], F32, tag="ssq")
    nc.vector.tensor_mul(ssq, s_sb, s_sb)
    sq = small.tile([P, 1], F32, tag="sq")
    nc.vector.reduce_sum(sq, ssq, axis=AX.X)
    # y = sqrt(sq) via exp(0.5 ln sq) + one Newton step
    lnq = small.tile([P, 1], F32, tag="lnq")
    nc.scalar.activation(lnq, sq, AF.Ln)
    y = small.tile([P, 1], F32, tag="y")
    nc.scalar.activation(y, lnq, AF.Exp, scale=0.5)
    ry = small.tile([P, 1], F32, tag="ry")
    nc.vector.reciprocal(ry, y)
    t1 = small.tile([P, 1], F32, tag="t1")
    nc.vector.tensor_mul(t1, sq, ry)
    nc.vector.tensor_add(t1, t1, y)
    nc.vector.tensor_scalar_mul(t1, t1, 0.5)
    d1 = small.tile([P, 1], F32, tag="d1")
    nc.vector.tensor_scalar_add(d1, sq, 1.0)
    rd = small.tile([P, 1], F32, tag="rd")
    nc.vector.reciprocal(rd, d1)
    sc = small.tile([P, 1], F32, tag="sc")
    nc.vector.tensor_mul(sc, t1, rd)
    v_sb = small.tile([P, F_SIZE], F32, tag="v_sb")
    nc.vector.tensor_scalar_mul(v_sb, s_sb, sc)
    if t == R:
        nc.sync.dma_start(v_out, v_sb)
        return
    if t == 1:
        nc.scalar.copy(w_acc, v_sb)
    else:
        nc.vector.tensor_add(w_acc, w_acc, v_sb)
    # broadcast w_acc [128 o, 16] -> w_rep [128 i-part, 2048] fp16 via DRAM
    w_bf = small.tile([P, F_SIZE], F16, tag="w_bf")
    nc.vector.tensor_copy(w_bf, w_acc)
    w_flat = dram.tile([RB], F16, tag="w_flat")
    nc.sync.dma_start(w_flat.rearrange("(p q) -> p q", p=P), w_bf)
    nc.sync.dma_start(
        w_rep, w_flat.unsqueeze(0).broadcast_to([P, RB]))


def _body(nc, tc, uh_d, v_out, R, rg):
    with ExitStack() as ctx:
        persist = ctx.enter_context(tc.tile_pool(name="persist", bufs=1))
        work = ctx.enter_context(tc.tile_pool(name="work", bufs=2))
        small = ctx.enter_context(tc.tile_pool(name="small", bufs=2))
        pspool = ctx.enter_context(tc.tile_pool(name="pspool", bufs=1, space="PSUM"))
        dram = ctx.enter_context(tc.tile_pool(name="dram", bufs=2, space="DRAM"))

        uh_sb = persist.tile([P, NBLK * RB], F16, name="uh_sb")
        w_rep = persist.tile([P, RB], F16, name="w_rep")
        c0 = persist.tile([P, P], F16, name="c0")
        nc.vector.memset(c0, 1.0 / OUT_NODES)
        e_all = persist.tile([P, NBLK * P], F16, name="e_all")
        den_all = persist.tile([P, NBLK], F32, name="den_all")
        rinv_all = persist.tile([P, NBLK], F32, name="rinv_all")
        w_acc = persist.tile([P, F_SIZE], F32, name="w_acc")
        ebias = persist.tile([P, 1], F32, name="ebias")
        nc.vector.memset(ebias, EXP_BIAS)
        # preload the joint exp+ln table set once: the insert pass then
        # sees every Exp/Ln covered and adds no further ACT_TABLE_LOADs
        nc.scalar.add_instruction(mybir.InstLoadActFuncSet(
            name="I-preload-act", act_func_set_id=6, ins=[], outs=[]))
        # dummy collective at start: pays the one-time CC stream setup
        # (~10us) during pass 1 instead of on pass 2's den AllReduce
        dum_in = dram.tile([CORES], F32, tag="dum_in")
        dum_out = dram.tile([CORES], F32, tag="dum_out")
        nc.gpsimd.collective_compute(
            "AllReduce", mybir.AluOpType.add, replica_groups=rg,
            ins=[dum_in.opt()], outs=[dum_out.opt()])

        # upload the uh shard once (fp16 from host)
        for b in range(NBLK):
            nc.sync.dma_start(uh_sb[:, b * RB:(b + 1) * RB], uh_d[b])

        for t in range(1, R + 1):
            s_ps = pspool.tile([P, RB], F32, tag="s_ps")
            if t > 1:
                # ---- sweep1: b, e, den partials; AR + sweep2 per half so
                # half 0's matmuls overlap half 1's sweep1 on DVE ----
                dnd0 = dram.tile([P * HB], F32, tag="dnd0")
                dno0 = dram.tile([P * HB], F32, tag="dno0")
                dnd1 = dram.tile([P * HB], F32, tag="dnd1")
                dno1 = dram.tile([P * HB], F32, tag="dno1")
                den_dr = [dnd0, dnd1]
                den_out = [dno0, dno1]
                for st in range(NBLK // DVE_STB):
                    g0 = st * DVE_STB * RB
                    sz = DVE_STB * RB
                    tm = work.tile([P, sz], F16, tag="tm_dve")
                    nc.vector.tensor_mul(
                        tm.rearrange("p (k r) -> p k r", r=RB),
                        uh_sb[:, g0:g0 + sz].rearrange(
                            "p (k r) -> p k r", r=RB),
                        w_rep[:, None, :].broadcast_to([P, DVE_STB, RB]),
                    )
                    b_sb = work.tile([P, DVE_STB * O_LOC], F32, tag="b_dve")
                    nc.vector.reduce_sum(
                        b_sb, tm.rearrange("p (o f) -> p o f", f=F_SIZE),
                        axis=AX.X)
                    for k in range(DVE_STB):
                        blk = st * DVE_STB + k
                        nc.scalar.activation(
                            e_all[:, blk * P:(blk + 1) * P],
                            b_sb[:, k * O_LOC:(k + 1) * O_LOC],
                            AF.Exp, bias=ebias,
                            accum_out=den_all[:, blk:blk + 1])
                    if (st + 1) * DVE_STB == HB:
                        nc.sync.dma_start(
                            den_dr[0].rearrange("(p q) -> p q", p=P),
                            den_all[:, :HB])
                        nc.gpsimd.collective_compute(
                            "AllReduce", mybir.AluOpType.add,
                            replica_groups=rg,
                            ins=[den_dr[0].opt()], outs=[den_out[0].opt()])
                        _sweep2_half(nc, small, pspool, s_ps, uh_sb,
                                     e_all, rinv_all, den_out[0], 0)
                nc.sync.dma_start(
                    den_dr[1].rearrange("(p q) -> p q", p=P),
                    den_all[:, HB:])
                nc.gpsimd.collective_compute(
                    "AllReduce", mybir.AluOpType.add, replica_groups=rg,
                    ins=[den_dr[1].opt()], outs=[den_out[1].opt()])
                _sweep2_half(nc, small, pspool, s_ps, uh_sb,
                             e_all, rinv_all, den_out[1], 1)
            else:
                # pass 1: chat uniform = 1/OUT -> pure PE pass.
                # Warm the PE clock first with full-width matmuls on the
                # first uploaded block (tiny MMs leave dispatch gaps and
                # never trip the HAM busy window).
                warm_ps = pspool.tile([P, MMW], F32, tag="warm_ps")
                for k in range(10):
                    _mm(nc, warm_ps, c0, uh_sb[:, 0:MMW],
                        start=True, stop=True, load=(k == 0))
                for blk in range(NBLK):
                    for c in range(RB // MMW):
                        _mm(nc, s_ps[:, c * MMW:(c + 1) * MMW],
                            c0,
                            uh_sb[:, blk * RB + c * MMW:
                                  blk * RB + (c + 1) * MMW],
                            start=(blk == 0), stop=(blk == NBLK - 1),
                            load=(blk == 0 and c == 0))
            # ---- diagonal extract: s[o,f] = ps[o, 16o+f] ----
            s_flat = work.tile([P, RB], F32, tag="s_flat", bufs=1)
            nc.scalar.copy(s_flat, s_ps)
            sdump = dram.tile([SDUMP], F32, tag="sdump")
            wview = sdump[0:P * WSTR].rearrange("(p q) -> p q", q=WSTR)
            nc.sync.dma_start(wview[:, 0:RB], s_flat)
            s_sb = small.tile([P, F_SIZE], F32, tag="s_sb")
            dview = sdump.rearrange("(p q) -> p q", q=DSTR)
            nc.sync.dma_start(s_sb, dview[:, 0:F_SIZE])
            _squash_and_w(nc, small, w_rep, s_sb, w_acc, t, R, v_out, dram)


def _build(routing_num: int):
    R = int(routing_num)
    assert R >= 1
    nc = bacc.Bacc(
        "TRN2", target_bir_lowering=False, debug=False, num_devices=CORES)
    uh = nc.dram_tensor("uh", [NBLK, P, RB], F16, kind="ExternalInput")
    v_out = nc.dram_tensor("v_out", [P, F_SIZE], F32, kind="ExternalOutput")
    rg = [list(range(CORES))]
    with tile.TileContext(nc) as tc:
        _body(nc, tc, uh.ap(), v_out.ap(), R, rg)
    _dedupe_ldweights(nc)
    nc.compile()
    return nc


_CACHE: dict = {}


def _get_nc(routing_num: int):
    R = int(routing_num)
    if R not in _CACHE:
        _CACHE[R] = _build(R)
    return _CACHE[R]


def _shard(u_hat: np.ndarray):
    uh = np.asarray(u_hat, dtype=np.float32)
    assert uh.shape == (IN_NODES * OUT_NODES, F_SIZE), uh.shape
    uh = uh.reshape(IN_NODES, OUT_NODES, F_SIZE)
    maps = []
    for c in range(CORES):
        sh = uh[:, c * O_LOC:(c + 1) * O_LOC, :].astype(np.float16)
        maps.append({"uh": np.ascontiguousarray(
            sh.reshape(NBLK, P, RB))})
    return maps


def run(u_hat, routing_num, trace=False):
    nc = _get_nc(routing_num)
    in_maps = _shard(u_hat)
    res = bass_utils.run_bass_kernel_spmd(
        nc, in_maps, core_ids=list(range(CORES)), trace=trace)
    return res


def kernel(u_hat, routing_num):
    res = run(u_hat, routing_num, trace=False)
    out = np.concatenate(
        [np.asarray(res.results[c]["v_out"], dtype=np.float32)
         for c in range(CORES)], axis=0)
    return out


# revision 49
# speedup vs baseline: 1.0614x; 1.0614x over previous
"""DGL capsule routing layer on 8 trn2 NeuronCores (Bass/Tile), v2.

Math per pass t (b is linear in uh, so b_t = uh . w_t with w_t = cumsum v):
    b[i,o] = sum_f uh[i,o,f] * w[o,f]
    e = exp(b - 3); den[i] = sum_o e  (AllReduce over o-shards)
    chat[i,o] = e * (1/den)
    s[o,f] = sum_i chat[i,o] * uh[i,o,f]; v = squash(s); w += v
Pass 1 has chat uniform = 1/OUT.

Sharding: OUT_NODES split across 8 cores (128 local o per core). Each core
holds its full uh shard [4096 i, 128 o, 16 f] in SBUF as fp16 (host converts,
upload not counted in HW time). Per-core per pass:
  sweep1 (passes>=2): per block of 128 i: tm = uh*w_rep (DVE/GpSimd TT fp16),
    b = reduce_f (DVE, f32), e = exp(b-3) on ACT with fused den accum.
    den partials AllReduce'd in two halves so the first half's matmuls
    overlap the second half's sweep1.
  sweep2: chat_blk = e_blk * rinv (DVE tensor_scalar, fp16) becomes the PE
    STATIONARY [128 i, 128 o]; moving = uh block [128, 2048] -> psum
    [128 o, 2048 (o,f)] accumulated over all 32 blocks. The wanted s[o,f]
    is the "diagonal" ps[o, 16o+f], extracted via a DRAM round-trip with a
    stride-2080 read AP. squash on [128 o, 16] layout, w broadcast via DRAM.
Output: core c returns v slice [128, 16]; host concatenates.
"""

import numpy as np
from contextlib import ExitStack

import concourse.bass as bass
import concourse.mybir as mybir
import concourse.tile as tile
from concourse import bacc
from concourse import bass_utils

F32 = mybir.dt.float32
F16 = mybir.dt.float16
AX = mybir.AxisListType
AF = mybir.ActivationFunctionType

IN_NODES, OUT_NODES, F_SIZE = 4096, 1024, 16
CORES = 8
O_LOC = OUT_NODES // CORES         # 128 local out-nodes per core
P = 128
NBLK = IN_NODES // P               # 32 i-blocks per core
RB = O_LOC * F_SIZE                # 2048 elems per i-row (128 o x 16 f)
DVE_STB = 4                        # sweep1 supertile = 4 blocks, all on DVE
                                   # (GpSimd TT steals the shared SBUF port
                                   # and DVE TT drops to ~1/8 rate when they
                                   # overlap -- measured; so no GpSimd tms)
EXP_BIAS = -3.0                    # exp(b-3): keeps e in fp16 range
MMW = 512                          # matmul moving width (one psum bank)
U32 = mybir.dt.uint32
ALU = mybir.AluOpType


def _mm(nc, out, lhsT, rhs, start, stop, load):
    return nc.tensor.matmul(out, lhsT, rhs, start=start, stop=stop,
                            skip_group_check=True)


def _ap_key(lowered):
    try:
        return str(lowered)
    except Exception:
        return repr(lowered)


def _dedupe_ldweights(nc):
    """Drop InstLdweights whose stationary AP equals the previous PE-queue
    load (the tile layer emits one per matmul; repeated stationaries keep
    the array contents valid). Runs after TileContext exit, before
    nc.compile(). Waits on a dropped load are merged into the next one's
    consumer matmul implicitly (observed: duplicate loads carry no syncs)."""
    PE = mybir.EngineType.PE
    for b in nc.main_func.blocks:
        out = []
        last_key = None
        for ins in b.instructions:
            if getattr(ins, "engine", None) != PE:
                out.append(ins)
                continue
            if isinstance(ins, mybir.InstLdweights):
                key = _ap_key(ins.ins[0])
                si = ins.sync_info
                has_sync = si is not None and (si.on_wait or si.on_update)
                if key == last_key and not has_sync:
                    continue
                last_key = key
                out.append(ins)
            elif isinstance(ins, mybir.InstMatmult):
                out.append(ins)
            else:
                last_key = None
                out.append(ins)
    # noqa: the slice assignment below replaces the list in place
        b.instructions[:] = out

# DRAM scratch for the diagonal extract: write rows at stride 2064,
# read the diagonal (o, 16o+f) at stride 2080.
WSTR = RB + F_SIZE                 # 2064
DSTR = RB + 2 * F_SIZE             # 2080
SDUMP = P * DSTR                   # 266240 floats


HB = NBLK // 2                     # blocks per AR half


def _sweep2_half(nc, small, pspool, s_ps, uh_sb, e_all, rinv_all,
                 den_out, h):
    """chat = e * (1/den) for half h. Everything here runs on ACT + PE +
    SP only, so it overlaps the other half's sweep1 on DVE. rinv is
    computed as exp(-ln(den)) on ACT (both in the preloaded table set)."""
    dsum = small.tile([P, HB], F32, tag=f"dsum{h}")
    nc.sync.dma_start(dsum, den_out.rearrange("(p q) -> p q", p=P))
    lnd = small.tile([P, HB], F32, tag=f"lnd{h}")
    nc.scalar.activation(lnd, dsum, AF.Ln)
    rinv_h = rinv_all[:, h * HB:(h + 1) * HB]
    nc.scalar.activation(rinv_h, lnd, AF.Exp, scale=-1.0)
    for k in range(HB):
        blk = h * HB + k
        ch = small.tile([P, P], F16, tag="ch", bufs=4)
        nc.scalar.mul(ch, e_all[:, blk * P:(blk + 1) * P],
                      rinv_all[:, blk:blk + 1])
        if k == 0:
            # HAM warm-up: ~4.5us of N=512 matmuls gated on this half's
            # first chat, so the PE clock is at 2.4 GHz for the real MMs
            warm_ps = pspool.tile([P, MMW], F32, tag="warm_ps")
            for w in range(20):
                _mm(nc, warm_ps, ch, uh_sb[:, 0:MMW],
                    start=True, stop=True, load=(w == 0))
        for c in range(RB // MMW):
            _mm(nc, s_ps[:, c * MMW:(c + 1) * MMW],
                ch,
                uh_sb[:, blk * RB + c * MMW:blk * RB + (c + 1) * MMW],
                start=(blk == 0), stop=(blk == NBLK - 1), load=(c == 0))


def _squash_and_w(nc, small, w_rep, s_sb, w_acc, t, R, v_out, dram):
    """s_sb [128 o, 16] f32 -> v; if not last pass, w_acc += v and broadcast
    w to w_rep [128, 2048] fp16 via DRAM."""
    ssq = small.tile([P, F_SIZE], F32, tag="ssq")
    nc.vector.tensor_mul(ssq, s_sb, s_sb)
    sq = small.tile([P, 1], F32, tag="sq")
    nc.vector.reduce_sum(sq, ssq, axis=AX.X)
    # y = sqrt(sq) via exp(0.5 ln sq) + one Newton step
    lnq = small.tile([P, 1], F32, tag="lnq")
    nc.scalar.activation(lnq, sq, AF.Ln)
    y = small.tile([P, 1], F32, tag="y")
    nc.scalar.activation(y, lnq, AF.Exp, scale=0.5)
    ry = small.tile([P, 1], F32, tag="ry")
    nc.vector.reciprocal(ry, y)
    t1 = small.tile([P, 1], F32, tag="t1")
    nc.vector.tensor_mul(t1, sq, ry)
    nc.vector.tensor_add(t1, t1, y)
    nc.vector.tensor_scalar_mul(t1, t1, 0.5)
    d1 = small.tile([P, 1], F32, tag="d1")
    nc.vector.tensor_scalar_add(d1, sq, 1.0)
    rd = small.tile([P, 1], F32, tag="rd")
    nc.vector.reciprocal(rd, d1)
    sc = small.tile([P, 1], F32, tag="sc")
    nc.vector.tensor_mul(sc, t1, rd)
    v_sb = small.tile([P, F_SIZE], F32, tag="v_sb")
    nc.vector.tensor_scalar_mul(v_sb, s_sb, sc)
    if t == R:
        nc.sync.dma_start(v_out, v_sb)
        return
    if t == 1:
        nc.scalar.copy(w_acc, v_sb)
    else:
        nc.vector.tensor_add(w_acc, w_acc, v_sb)
    # broadcast w_acc [128 o, 16] -> w_rep [128 i-part, 2048] fp16 via DRAM
    w_bf = small.tile([P, F_SIZE], F16, tag="w_bf")
    nc.vector.tensor_copy(w_bf, w_acc)
    w_flat = dram.tile([RB], F16, tag="w_flat")
    nc.sync.dma_start(w_flat.rearrange("(p q) -> p q", p=P), w_bf)
    nc.sync.dma_start(
        w_rep, w_flat.unsqueeze(0).broadcast_to([P, RB]))


def _body(nc, tc, uh_d, v_out, R, rg):
    with ExitStack() as ctx:
        persist = ctx.enter_context(tc.tile_pool(name="persist", bufs=1))
        work = ctx.enter_context(tc.tile_pool(name="work", bufs=2))
        small = ctx.enter_context(tc.tile_pool(name="small", bufs=2))
        pspool = ctx.enter_context(tc.tile_pool(name="pspool", bufs=1, space="PSUM"))
        dram = ctx.enter_context(tc.tile_pool(name="dram", bufs=2, space="DRAM"))

        uh_sb = persist.tile([P, NBLK * RB], F16, name="uh_sb")
        w_rep = persist.tile([P, RB], F16, name="w_rep")
        c0 = persist.tile([P, P], F16, name="c0")
        nc.vector.memset(c0, 1.0 / OUT_NODES)
        e_all = persist.tile([P, NBLK * P], F16, name="e_all")
        den_all = persist.tile([P, NBLK], F32, name="den_all")
        rinv_all = persist.tile([P, NBLK], F32, name="rinv_all")
        w_acc = persist.tile([P, F_SIZE], F32, name="w_acc")
        ebias = persist.tile([P, 1], F32, name="ebias")
        nc.vector.memset(ebias, EXP_BIAS)
        # preload the joint exp+ln table set once: the insert pass then
        # sees every Exp/Ln covered and adds no further ACT_TABLE_LOADs
        nc.scalar.add_instruction(mybir.InstLoadActFuncSet(
            name="I-preload-act", act_func_set_id=6, ins=[], outs=[]))
        # dummy collective at start: pays the one-time CC stream setup
        # (~10us) during pass 1 instead of on pass 2's den AllReduce
        dum_in = dram.tile([CORES], F32, tag="dum_in")
        dum_out = dram.tile([CORES], F32, tag="dum_out")
        nc.gpsimd.collective_compute(
            "AllReduce", mybir.AluOpType.add, replica_groups=rg,
            ins=[dum_in.opt()], outs=[dum_out.opt()])

        # upload the uh shard once (fp16 from host)
        for b in range(NBLK):
            nc.sync.dma_start(uh_sb[:, b * RB:(b + 1) * RB], uh_d[b])

        for t in range(1, R + 1):
            s_ps = pspool.tile([P, RB], F32, tag="s_ps")
            if t > 1:
                # ---- sweep1: b, e, den partials; AR + sweep2 per half so
                # half 0's matmuls overlap half 1's sweep1 on DVE ----
                dnd0 = dram.tile([P * HB], F32, tag="dnd0")
                dno0 = dram.tile([P * HB], F32, tag="dno0")
                dnd1 = dram.tile([P * HB], F32, tag="dnd1")
                dno1 = dram.tile([P * HB], F32, tag="dno1")
                den_dr = [dnd0, dnd1]
                den_out = [dno0, dno1]
                for st in range(NBLK // DVE_STB):
                    g0 = st * DVE_STB * RB
                    sz = DVE_STB * RB
                    tm = work.tile([P, sz], F16, tag="tm_dve")
                    nc.vector.tensor_mul(
                        tm.rearrange("p (k r) -> p k r", r=RB),
                        uh_sb[:, g0:g0 + sz].rearrange(
                            "p (k r) -> p k r", r=RB),
                        w_rep[:, None, :].broadcast_to([P, DVE_STB, RB]),
                    )
                    b_sb = work.tile([P, DVE_STB * O_LOC], F32, tag="b_dve")
                    nc.vector.reduce_sum(
                        b_sb, tm.rearrange("p (o f) -> p o f", f=F_SIZE),
                        axis=AX.X)
                    for k in range(DVE_STB):
                        blk = st * DVE_STB + k
                        nc.scalar.activation(
                            e_all[:, blk * P:(blk + 1) * P],
                            b_sb[:, k * O_LOC:(k + 1) * O_LOC],
                            AF.Exp, bias=ebias,
                            accum_out=den_all[:, blk:blk + 1])
                    if (st + 1) * DVE_STB == HB:
                        nc.sync.dma_start(
                            den_dr[0].rearrange("(p q) -> p q", p=P),
                            den_all[:, :HB])
                        nc.gpsimd.collective_compute(
                            "AllReduce", mybir.AluOpType.add,
                            replica_groups=rg,
                            ins=[den_dr[0].opt()], outs=[den_out[0].opt()])
                        _sweep2_half(nc, small, pspool, s_ps, uh_sb,
                                     e_all, rinv_all, den_out[0], 0)
                nc.sync.dma_start(
                    den_dr[1].rearrange("(p q) -> p q", p=P),
                    den_all[:, HB:])
                nc.gpsimd.collective_compute(
                    "AllReduce", mybir.AluOpType.add, replica_groups=rg,
                    ins=[den_dr[1].opt()], outs=[den_out[1].opt()])
                _sweep2_half(nc, small, pspool, s_ps, uh_sb,
                             e_all, rinv_all, den_out[1], 1)
            else:
                # pass 1: chat uniform = 1/OUT -> pure PE pass.
                # Warm the PE clock first with full-width matmuls on the
                # first uploaded block (tiny MMs leave dispatch gaps and
                # never trip the HAM busy window).
                warm_ps = pspool.tile([P, MMW], F32, tag="warm_ps")
                for k in range(20):
                    _mm(nc, warm_ps, c0, uh_sb[:, 0:MMW],
                        start=True, stop=True, load=(k == 0))
                for blk in range(NBLK):
                    for c in range(RB // MMW):
                        _mm(nc, s_ps[:, c * MMW:(c + 1) * MMW],
                            c0,
                            uh_sb[:, blk * RB + c * MMW:
                                  blk * RB + (c + 1) * MMW],
                            start=(blk == 0), stop=(blk == NBLK - 1),
                            load=(blk == 0 and c == 0))
            # ---- diagonal extract: s[o,f] = ps[o, 16o+f] ----
            s_flat = work.tile([P, RB], F32, tag="s_flat", bufs=1)
            nc.scalar.copy(s_flat, s_ps)
            sdump = dram.tile([SDUMP], F32, tag="sdump")
            wview = sdump[0:P * WSTR].rearrange("(p q) -> p q", q=WSTR)
            nc.sync.dma_start(wview[:, 0:RB], s_flat)
            s_sb = small.tile([P, F_SIZE], F32, tag="s_sb")
            dview = sdump.rearrange("(p q) -> p q", q=DSTR)
            nc.sync.dma_start(s_sb, dview[:, 0:F_SIZE])
            _squash_and_w(nc, small, w_rep, s_sb, w_acc, t, R, v_out, dram)


def _build(routing_num: int):
    R = int(routing_num)
    assert R >= 1
    nc = bacc.Bacc(
        "TRN2", target_bir_lowering=False, debug=False, num_devices=CORES)
    uh = nc.dram_tensor("uh", [NBLK, P, RB], F16, kind="ExternalInput")
    v_out = nc.dram_tensor("v_out", [P, F_SIZE], F32, kind="ExternalOutput")
    rg = [list(range(CORES))]
    with tile.TileContext(nc) as tc:
        _body(nc, tc, uh.ap(), v_out.ap(), R, rg)
    _dedupe_ldweights(nc)
    nc.compile()
    return nc


_CACHE: dict = {}


def _get_nc(routing_num: int):
    R = int(routing_num)
    if R not in _CACHE:
        _CACHE[R] = _build(R)
    return _CACHE[R]


def _shard(u_hat: np.ndarray):
    uh = np.asarray(u_hat, dtype=np.float32)
    assert uh.shape == (IN_NODES * OUT_NODES, F_SIZE), uh.shape
    uh = uh.reshape(IN_NODES, OUT_NODES, F_SIZE)
    maps = []
    for c in range(CORES):
        sh = uh[:, c * O_LOC:(c + 1) * O_LOC, :].astype(np.float16)
        maps.append({"uh": np.ascontiguousarray(
            sh.reshape(NBLK, P, RB))})
    return maps


def run(u_hat, routing_num, trace=False):
    nc = _get_nc(routing_num)
    in_maps = _shard(u_hat)
    res = bass_utils.run_bass_kernel_spmd(
        nc, in_maps, core_ids=list(range(CORES)), trace=trace)
    return res


def kernel(u_hat, routing_num):
    res = run(u_hat, routing_num, trace=False)
    out = np.concatenate(
        [np.asarray(res.results[c]["v_out"], dtype=np.float32)
         for c in range(CORES)], axis=0)
    return out


# revision 50
# speedup vs baseline: 1.0743x; 1.0121x over previous
"""DGL capsule routing layer on 8 trn2 NeuronCores (Bass/Tile), v2.

Math per pass t (b is linear in uh, so b_t = uh . w_t with w_t = cumsum v):
    b[i,o] = sum_f uh[i,o,f] * w[o,f]
    e = exp(b - 3); den[i] = sum_o e  (AllReduce over o-shards)
    chat[i,o] = e * (1/den)
    s[o,f] = sum_i chat[i,o] * uh[i,o,f]; v = squash(s); w += v
Pass 1 has chat uniform = 1/OUT.

Sharding: OUT_NODES split across 8 cores (128 local o per core). Each core
holds its full uh shard [4096 i, 128 o, 16 f] in SBUF as fp16 (host converts,
upload not counted in HW time). Per-core per pass:
  sweep1 (passes>=2): per block of 128 i: tm = uh*w_rep (DVE/GpSimd TT fp16),
    b = reduce_f (DVE, f32), e = exp(b-3) on ACT with fused den accum.
    den partials AllReduce'd in two halves so the first half's matmuls
    overlap the second half's sweep1.
  sweep2: chat_blk = e_blk * rinv (DVE tensor_scalar, fp16) becomes the PE
    STATIONARY [128 i, 128 o]; moving = uh block [128, 2048] -> psum
    [128 o, 2048 (o,f)] accumulated over all 32 blocks. The wanted s[o,f]
    is the "diagonal" ps[o, 16o+f], extracted via a DRAM round-trip with a
    stride-2080 read AP. squash on [128 o, 16] layout, w broadcast via DRAM.
Output: core c returns v slice [128, 16]; host concatenates.
"""

import numpy as np
from contextlib import ExitStack

import concourse.bass as bass
import concourse.mybir as mybir
import concourse.tile as tile
from concourse import bacc
from concourse import bass_utils

F32 = mybir.dt.float32
F16 = mybir.dt.float16
AX = mybir.AxisListType
AF = mybir.ActivationFunctionType

IN_NODES, OUT_NODES, F_SIZE = 4096, 1024, 16
CORES = 8
O_LOC = OUT_NODES // CORES         # 128 local out-nodes per core
P = 128
NBLK = IN_NODES // P               # 32 i-blocks per core
RB = O_LOC * F_SIZE                # 2048 elems per i-row (128 o x 16 f)
DVE_STB = 4                        # sweep1 supertile = 4 blocks, all on DVE
                                   # (GpSimd TT steals the shared SBUF port
                                   # and DVE TT drops to ~1/8 rate when they
                                   # overlap -- measured; so no GpSimd tms)
EXP_BIAS = -3.0                    # exp(b-3): keeps e in fp16 range
MMW = 512                          # matmul moving width (one psum bank)
U32 = mybir.dt.uint32
ALU = mybir.AluOpType


def _mm(nc, out, lhsT, rhs, start, stop, load):
    return nc.tensor.matmul(out, lhsT, rhs, start=start, stop=stop,
                            skip_group_check=True)


def _ap_key(lowered):
    try:
        return str(lowered)
    except Exception:
        return repr(lowered)


def _dedupe_ldweights(nc):
    """Drop InstLdweights whose stationary AP equals the previous PE-queue
    load (the tile layer emits one per matmul; repeated stationaries keep
    the array contents valid). Runs after TileContext exit, before
    nc.compile(). Waits on a dropped load are merged into the next one's
    consumer matmul implicitly (observed: duplicate loads carry no syncs)."""
    PE = mybir.EngineType.PE
    for b in nc.main_func.blocks:
        out = []
        last_key = None
        for ins in b.instructions:
            if getattr(ins, "engine", None) != PE:
                out.append(ins)
                continue
            if isinstance(ins, mybir.InstLdweights):
                key = _ap_key(ins.ins[0])
                si = ins.sync_info
                has_sync = si is not None and (si.on_wait or si.on_update)
                if key == last_key and not has_sync:
                    continue
                last_key = key
                out.append(ins)
            elif isinstance(ins, mybir.InstMatmult):
                out.append(ins)
            else:
                last_key = None
                out.append(ins)
    # noqa: the slice assignment below replaces the list in place
        b.instructions[:] = out

# DRAM scratch for the diagonal extract: write rows at stride 2064,
# read the diagonal (o, 16o+f) at stride 2080.
WSTR = RB + F_SIZE                 # 2064
DSTR = RB + 2 * F_SIZE             # 2080
SDUMP = P * DSTR                   # 266240 floats


HB = NBLK // 2                     # blocks per AR half


def _sweep2_half(nc, small, pspool, s_ps, uh_sb, e_all, rinv_all,
                 den_out, h):
    """chat = e * (1/den) for half h. Everything here runs on ACT + PE +
    SP only, so it overlaps the other half's sweep1 on DVE. rinv is
    computed as exp(-ln(den)) on ACT (both in the preloaded table set)."""
    dsum = small.tile([P, HB], F32, tag=f"dsum{h}")
    nc.sync.dma_start(dsum, den_out.rearrange("(p q) -> p q", p=P))
    lnd = small.tile([P, HB], F32, tag=f"lnd{h}")
    nc.scalar.activation(lnd, dsum, AF.Ln)
    rinv_h = rinv_all[:, h * HB:(h + 1) * HB]
    nc.scalar.activation(rinv_h, lnd, AF.Exp, scale=-1.0)
    for k in range(HB):
        blk = h * HB + k
        ch = small.tile([P, P], F16, tag="ch", bufs=4)
        nc.scalar.mul(ch, e_all[:, blk * P:(blk + 1) * P],
                      rinv_all[:, blk:blk + 1])
        if k == 0:
            # HAM warm-up: ~4.5us of N=512 matmuls gated on this half's
            # first chat, so the PE clock is at 2.4 GHz for the real MMs
            warm_ps = pspool.tile([P, MMW], F32, tag="warm_ps")
            for w in range(20):
                _mm(nc, warm_ps, ch, uh_sb[:, 0:MMW],
                    start=True, stop=True, load=(w == 0))
        for c in range(RB // MMW):
            _mm(nc, s_ps[:, c * MMW:(c + 1) * MMW],
                ch,
                uh_sb[:, blk * RB + c * MMW:blk * RB + (c + 1) * MMW],
                start=(blk == 0), stop=(blk == NBLK - 1), load=(c == 0))


def _squash_and_w(nc, small, w_rep, s_sb, w_acc, t, R, v_out, dram):
    """s_sb [128 o, 16] f32 -> v; if not last pass, w_acc += v and broadcast
    w to w_rep [128, 2048] fp16 via DRAM."""
    ssq = small.tile([P, F_SIZE], F32, tag="ssq")
    nc.vector.tensor_mul(ssq, s_sb, s_sb)
    sq = small.tile([P, 1], F32, tag="sq")
    nc.vector.reduce_sum(sq, ssq, axis=AX.X)
    # y = sqrt(sq) via exp(0.5 ln sq) + one Newton step
    lnq = small.tile([P, 1], F32, tag="lnq")
    nc.scalar.activation(lnq, sq, AF.Ln)
    y = small.tile([P, 1], F32, tag="y")
    nc.scalar.activation(y, lnq, AF.Exp, scale=0.5)
    ry = small.tile([P, 1], F32, tag="ry")
    nc.vector.reciprocal(ry, y)
    t1 = small.tile([P, 1], F32, tag="t1")
    nc.vector.tensor_mul(t1, sq, ry)
    nc.vector.tensor_add(t1, t1, y)
    nc.vector.tensor_scalar_mul(t1, t1, 0.5)
    d1 = small.tile([P, 1], F32, tag="d1")
    nc.vector.tensor_scalar_add(d1, sq, 1.0)
    rd = small.tile([P, 1], F32, tag="rd")
    nc.vector.reciprocal(rd, d1)
    sc = small.tile([P, 1], F32, tag="sc")
    nc.vector.tensor_mul(sc, t1, rd)
    v_sb = small.tile([P, F_SIZE], F32, tag="v_sb")
    nc.vector.tensor_scalar_mul(v_sb, s_sb, sc)
    if t == R:
        nc.sync.dma_start(v_out, v_sb)
        return
    if t == 1:
        nc.scalar.copy(w_acc, v_sb)
    else:
        nc.vector.tensor_add(w_acc, w_acc, v_sb)
    # broadcast w_acc [128 o, 16] -> w_rep [128 i-part, 2048] fp16 via DRAM
    w_bf = small.tile([P, F_SIZE], F16, tag="w_bf")
    nc.vector.tensor_copy(w_bf, w_acc)
    w_flat = dram.tile([RB], F16, tag="w_flat")
    nc.sync.dma_start(w_flat.rearrange("(p q) -> p q", p=P), w_bf)
    nc.sync.dma_start(
        w_rep, w_flat.unsqueeze(0).broadcast_to([P, RB]))


def _body(nc, tc, uh_d, v_out, R, rg):
    with ExitStack() as ctx:
        persist = ctx.enter_context(tc.tile_pool(name="persist", bufs=1))
        work = ctx.enter_context(tc.tile_pool(name="work", bufs=2))
        small = ctx.enter_context(tc.tile_pool(name="small", bufs=2))
        pspool = ctx.enter_context(tc.tile_pool(name="pspool", bufs=1, space="PSUM"))
        dram = ctx.enter_context(tc.tile_pool(name="dram", bufs=2, space="DRAM"))

        uh_sb = persist.tile([P, NBLK * RB], F16, name="uh_sb")
        w_rep = persist.tile([P, RB], F16, name="w_rep")
        c0 = persist.tile([P, P], F16, name="c0")
        nc.vector.memset(c0, 1.0 / OUT_NODES)
        e_all = persist.tile([P, NBLK * P], F16, name="e_all")
        den_all = persist.tile([P, NBLK], F32, name="den_all")
        rinv_all = persist.tile([P, NBLK], F32, name="rinv_all")
        w_acc = persist.tile([P, F_SIZE], F32, name="w_acc")
        ebias = persist.tile([P, 1], F32, name="ebias")
        nc.vector.memset(ebias, EXP_BIAS)
        # preload the joint exp+ln table set once: the insert pass then
        # sees every Exp/Ln covered and adds no further ACT_TABLE_LOADs
        nc.scalar.add_instruction(mybir.InstLoadActFuncSet(
            name="I-preload-act", act_func_set_id=6, ins=[], outs=[]))
        # dummy collective at start: pays the one-time CC stream setup
        # (~10us) during pass 1 instead of on pass 2's den AllReduce
        dum_in = dram.tile([CORES], F32, tag="dum_in")
        dum_out = dram.tile([CORES], F32, tag="dum_out")
        nc.gpsimd.collective_compute(
            "AllReduce", mybir.AluOpType.add, replica_groups=rg,
            ins=[dum_in.opt()], outs=[dum_out.opt()])

        # upload the uh shard once (fp16 from host)
        for b in range(NBLK):
            nc.sync.dma_start(uh_sb[:, b * RB:(b + 1) * RB], uh_d[b])

        for t in range(1, R + 1):
            s_ps = pspool.tile([P, RB], F32, tag="s_ps")
            if t > 1:
                # ---- sweep1: b, e, den partials; AR + sweep2 per half so
                # half 0's matmuls overlap half 1's sweep1 on DVE ----
                dnd0 = dram.tile([P * HB], F32, tag="dnd0")
                dno0 = dram.tile([P * HB], F32, tag="dno0")
                dnd1 = dram.tile([P * HB], F32, tag="dnd1")
                dno1 = dram.tile([P * HB], F32, tag="dno1")
                den_dr = [dnd0, dnd1]
                den_out = [dno0, dno1]
                # supertile spans: the last tile of each half is split in
                # two so the half's final den partial (which gates the AR
                # trigger) lands one small reduce earlier
                spans = [(0, 4), (4, 4), (8, 4), (12, 2), (14, 2),
                         (16, 4), (20, 4), (24, 4), (28, 2), (30, 2)]
                for b0, nb in spans:
                    g0 = b0 * RB
                    sz = nb * RB
                    tm = work.tile([P, sz], F16, tag=f"tm{nb}")
                    nc.vector.tensor_mul(
                        tm.rearrange("p (k r) -> p k r", r=RB),
                        uh_sb[:, g0:g0 + sz].rearrange(
                            "p (k r) -> p k r", r=RB),
                        w_rep[:, None, :].broadcast_to([P, nb, RB]),
                    )
                    b_sb = work.tile([P, nb * O_LOC], F32, tag=f"b{nb}")
                    nc.vector.reduce_sum(
                        b_sb, tm.rearrange("p (o f) -> p o f", f=F_SIZE),
                        axis=AX.X)
                    for k in range(nb):
                        blk = b0 + k
                        nc.scalar.activation(
                            e_all[:, blk * P:(blk + 1) * P],
                            b_sb[:, k * O_LOC:(k + 1) * O_LOC],
                            AF.Exp, bias=ebias,
                            accum_out=den_all[:, blk:blk + 1])
                    if b0 + nb == HB:
                        nc.sync.dma_start(
                            den_dr[0].rearrange("(p q) -> p q", p=P),
                            den_all[:, :HB])
                        nc.gpsimd.collective_compute(
                            "AllReduce", mybir.AluOpType.add,
                            replica_groups=rg,
                            ins=[den_dr[0].opt()], outs=[den_out[0].opt()])
                        _sweep2_half(nc, small, pspool, s_ps, uh_sb,
                                     e_all, rinv_all, den_out[0], 0)
                nc.sync.dma_start(
                    den_dr[1].rearrange("(p q) -> p q", p=P),
                    den_all[:, HB:])
                nc.gpsimd.collective_compute(
                    "AllReduce", mybir.AluOpType.add, replica_groups=rg,
                    ins=[den_dr[1].opt()], outs=[den_out[1].opt()])
                _sweep2_half(nc, small, pspool, s_ps, uh_sb,
                             e_all, rinv_all, den_out[1], 1)
            else:
                # pass 1: chat uniform = 1/OUT -> pure PE pass.
                # Warm the PE clock first with full-width matmuls on the
                # first uploaded block (tiny MMs leave dispatch gaps and
                # never trip the HAM busy window).
                warm_ps = pspool.tile([P, MMW], F32, tag="warm_ps")
                for k in range(20):
                    _mm(nc, warm_ps, c0, uh_sb[:, 0:MMW],
                        start=True, stop=True, load=(k == 0))
                for blk in range(NBLK):
                    for c in range(RB // MMW):
                        _mm(nc, s_ps[:, c * MMW:(c + 1) * MMW],
                            c0,
                            uh_sb[:, blk * RB + c * MMW:
                                  blk * RB + (c + 1) * MMW],
                            start=(blk == 0), stop=(blk == NBLK - 1),
                            load=(blk == 0 and c == 0))
            # ---- diagonal extract: s[o,f] = ps[o, 16o+f] ----
            s_flat = work.tile([P, RB], F32, tag="s_flat", bufs=1)
            nc.scalar.copy(s_flat, s_ps)
            sdump = dram.tile([SDUMP], F32, tag="sdump")
            wview = sdump[0:P * WSTR].rearrange("(p q) -> p q", q=WSTR)
            nc.sync.dma_start(wview[:, 0:RB], s_flat)
            s_sb = small.tile([P, F_SIZE], F32, tag="s_sb")
            dview = sdump.rearrange("(p q) -> p q", q=DSTR)
            nc.sync.dma_start(s_sb, dview[:, 0:F_SIZE])
            _squash_and_w(nc, small, w_rep, s_sb, w_acc, t, R, v_out, dram)


def _build(routing_num: int):
    R = int(routing_num)
    assert R >= 1
    nc = bacc.Bacc(
        "TRN2", target_bir_lowering=False, debug=False, num_devices=CORES)
    uh = nc.dram_tensor("uh", [NBLK, P, RB], F16, kind="ExternalInput")
    v_out = nc.dram_tensor("v_out", [P, F_SIZE], F32, kind="ExternalOutput")
    rg = [list(range(CORES))]
    with tile.TileContext(nc) as tc:
        _body(nc, tc, uh.ap(), v_out.ap(), R, rg)
    _dedupe_ldweights(nc)
    nc.compile()
    return nc


_CACHE: dict = {}


def _get_nc(routing_num: int):
    R = int(routing_num)
    if R not in _CACHE:
        _CACHE[R] = _build(R)
    return _CACHE[R]


def _shard(u_hat: np.ndarray):
    uh = np.asarray(u_hat, dtype=np.float32)
    assert uh.shape == (IN_NODES * OUT_NODES, F_SIZE), uh.shape
    uh = uh.reshape(IN_NODES, OUT_NODES, F_SIZE)
    maps = []
    for c in range(CORES):
        sh = uh[:, c * O_LOC:(c + 1) * O_LOC, :].astype(np.float16)
        maps.append({"uh": np.ascontiguousarray(
            sh.reshape(NBLK, P, RB))})
    return maps


def run(u_hat, routing_num, trace=False):
    nc = _get_nc(routing_num)
    in_maps = _shard(u_hat)
    res = bass_utils.run_bass_kernel_spmd(
        nc, in_maps, core_ids=list(range(CORES)), trace=trace)
    return res


def kernel(u_hat, routing_num):
    res = run(u_hat, routing_num, trace=False)
    out = np.concatenate(
        [np.asarray(res.results[c]["v_out"], dtype=np.float32)
         for c in range(CORES)], axis=0)
    return out
